# revision 5
# baseline (speedup 1.0000x reference)
"""Trainium2 Bass kernel for PlayerSelectionNetwork (16-agent GRU + MLP head).

Strategy (8 NeuronCores, data-parallel over batch):
  - Each core processes B=2048 rows of x (16384/8).
  - Feature-major GRU: hidden state h kept as (features, batch) tiles, agents
    packed in pairs on the 128 SBUF partitions (agent 2p -> partitions 0:64,
    agent 2p+1 -> 64:128).
  - Gate matmuls are K=128 block-diagonal: lhsT holds both agents' recurrent
    weights on disjoint row/col blocks, so one matmul produces a pair-packed
    gate tile (r_a; r_b) directly in PSUM.
  - Input projections x@Wi are folded into the same PSUM accumulation via a
    second block-diagonal matmul against a feature-major x slab (two
    timesteps per 128-row slab).
  - Elementwise gate math runs pair-packed on ACT (sigmoid/tanh with fused
    per-partition bias) / DVE / GPSIMD, balanced across the three engines.
  - MLP head stays feature-major; final (15, B) activations are transposed
    to batch-major with the PE transpose and DMA'd out contiguously.

Weights are pre-packed on the host (block-diagonal layout, bf16) and shipped
as extra kernel inputs - they are tiny and replicated on all cores.
"""

import numpy as np
import ml_dtypes

# Model constants (match the reference problem definition).
B_FULL = 16384
N_CORES = 8
B = B_FULL // N_CORES  # per-core batch
T_OBS = 10
N_AGENTS = 16
INPUT_DIM = 4
H = 64
HID1 = 512
HID2 = 256
M_OUT = 15
F_IN = T_OBS * N_AGENTS * INPUT_DIM  # 640
FEAT = N_AGENTS * H  # 1024

CHUNK = 512  # moving free dim per matmul / psum tile
NPAIR = N_AGENTS // 2  # 8
NSLAB = F_IN // 128  # 5 (two timesteps per slab)


def build_nc(Bc=B, chunk=CHUNK, bhn_zero=True):
    """Build + compile the single-core Bass program (SPMD-replicated)."""
    import concourse.bacc as bacc
    import concourse.mybir as mybir
    import concourse.tile as tile
    from contextlib import ExitStack

    f32 = mybir.dt.float32
    bf16 = mybir.dt.bfloat16
    AFT = mybir.ActivationFunctionType

    nch = Bc // chunk
    half = Bc // 2
    nbt = Bc // 128

    nc = bacc.Bacc("TRN2", target_bir_lowering=False, debug=False)

    x = nc.dram_tensor("x", (Bc, F_IN), f32, kind="ExternalInput").ap()
    WHL = nc.dram_tensor("WHL", (3, NPAIR, 128, 128), bf16, kind="ExternalInput").ap()
    WXL = nc.dram_tensor("WXL", (3, NPAIR, 2, 128, 128), bf16, kind="ExternalInput").ap()
    W1L = nc.dram_tensor("W1L", (FEAT // 128, HID1 // 128, 128, 128), bf16, kind="ExternalInput").ap()
    W2L = nc.dram_tensor("W2L", (HID1 // 128, HID2 // 128, 128, 128), bf16, kind="ExternalInput").ap()
    WOL = nc.dram_tensor("WOL", (HID2 // 128, 128, M_OUT), bf16, kind="ExternalInput").ap()
    BI = nc.dram_tensor("BI", (3, NPAIR, 128, 1), f32, kind="ExternalInput").ap()
    BHN = nc.dram_tensor("BHN", (NPAIR, 128, 1), f32, kind="ExternalInput").ap()
    B1 = nc.dram_tensor("B1", (HID1 // 128, 128, 1), f32, kind="ExternalInput").ap()
    B2 = nc.dram_tensor("B2", (HID2 // 128, 128, 1), f32, kind="ExternalInput").ap()
    BOUT = nc.dram_tensor("BOUT", (M_OUT, 1), f32, kind="ExternalInput").ap()
    IDT = nc.dram_tensor("IDT", (M_OUT, M_OUT), f32, kind="ExternalInput").ap()
    out = nc.dram_tensor("out", (Bc, M_OUT), f32, kind="ExternalOutput").ap()

    with tile.TileContext(nc) as tc, ExitStack() as ctx:
        # ---- persistent weight / bias tiles ----
        wp = ctx.enter_context(tc.tile_pool(name="weights", bufs=1))
        whl = [[wp.tile([128, 128], bf16, name=f"whl{g}_{p}") for p in range(NPAIR)] for g in range(3)]
        wxl = [[[wp.tile([128, 128], bf16, name=f"wxl{g}_{p}_{q}") for q in range(2)] for p in range(NPAIR)] for g in range(3)]
        w1l = [[wp.tile([128, 128], bf16, name=f"w1l{k}_{m}") for m in range(HID1 // 128)] for k in range(FEAT // 128)]
        w2l = [[wp.tile([128, 128], bf16, name=f"w2l{k}_{m}") for m in range(HID2 // 128)] for k in range(HID1 // 128)]
        wol = [wp.tile([128, M_OUT], bf16, name=f"wol{k}") for k in range(HID2 // 128)]
        bi_sb = [[wp.tile([128, 1], f32, name=f"bi{g}_{p}") for p in range(NPAIR)] for g in range(3)]
        bhn_sb = [wp.tile([128, 1], f32, name=f"bhn{p}") for p in range(NPAIR)]
        b1_sb = [wp.tile([128, 1], f32, name=f"b1_{m}") for m in range(HID1 // 128)]
        b2_sb = [wp.tile([128, 1], f32, name=f"b2_{m}") for m in range(HID2 // 128)]
        bout_sb = wp.tile([M_OUT, 1], f32, name="bout_sb")
        ident_sb = wp.tile([M_OUT, M_OUT], f32, name="ident_sb")

        for g in range(3):
            for p in range(NPAIR):
                nc.sync.dma_start(whl[g][p][:], WHL[g, p])
                nc.sync.dma_start(bi_sb[g][p][:], BI[g, p])
                for q in range(2):
                    nc.sync.dma_start(wxl[g][p][q][:], WXL[g, p, q])
        for p in range(NPAIR):
            nc.sync.dma_start(bhn_sb[p][:], BHN[p])
        for k in range(FEAT // 128):
            for m in range(HID1 // 128):
                nc.sync.dma_start(w1l[k][m][:], W1L[k, m])
        for k in range(HID1 // 128):
            for m in range(HID2 // 128):
                nc.sync.dma_start(w2l[k][m][:], W2L[k, m])
        for k in range(HID2 // 128):
            nc.sync.dma_start(wol[k][:], WOL[k])
        for m in range(HID1 // 128):
            nc.sync.dma_start(b1_sb[m][:], B1[m])
        for m in range(HID2 // 128):
            nc.sync.dma_start(b2_sb[m][:], B2[m])
        nc.sync.dma_start(bout_sb[:], BOUT[:])
        nc.sync.dma_start(ident_sb[:], IDT[:])

        # ---- x -> feature-major bf16 slabs (two timesteps per slab) ----
        sp = ctx.enter_context(tc.tile_pool(name="slabs", bufs=1))
        slab = [sp.tile([128, Bc], bf16, name=f"slab{k}") for k in range(NSLAB)]
        with tc.tile_pool(name="xstage", bufs=3) as xsp, tc.tile_pool(name="xbf", bufs=3) as xbp:
            for bt in range(nbt):
                xf = xsp.tile([128, F_IN], f32, tag="xf", name=f"xf{bt}")
                nc.sync.dma_start(xf[:], x[128 * bt:128 * bt + 128, :])
                xb = xbp.tile([128, F_IN], bf16, tag="xb", name=f"xb{bt}")
                nc.vector.tensor_copy(xb[:], xf[:])
                for k in range(NSLAB):
                    nc.sync.dma_start(
                        slab[k][:, 128 * bt:128 * bt + 128],
                        xb[:, 128 * k:128 * k + 128],
                        transpose=True,
                    )

        # ---- GRU ----
        hp = ctx.enter_context(tc.tile_pool(name="h", bufs=2))
        znp = ctx.enter_context(tc.tile_pool(name="zn", bufs=4))
        gp = ctx.enter_context(tc.tile_pool(name="gates", bufs=3))
        dep = ctx.enter_context(tc.tile_pool(name="de", bufs=2))
        gru_psum = ExitStack()
        ppr = gru_psum.enter_context(tc.tile_pool(name="ppr", bufs=2, space="PSUM"))
        ppz = gru_psum.enter_context(tc.tile_pool(name="ppz", bufs=2, space="PSUM"))
        pphn = gru_psum.enter_context(tc.tile_pool(name="pphn", bufs=2, space="PSUM"))
        ppxn = gru_psum.enter_context(tc.tile_pool(name="ppxn", bufs=2, space="PSUM"))

        h = []
        for p in range(NPAIR):
            t0 = hp.tile([128, Bc], bf16, tag=f"h{p}", name=f"h_init{p}")
            nc.gpsimd.memset(t0[:], 0.0)
            h.append(t0)

        for t in range(T_OBS):
            k, par = t // 2, t % 2
            hnew = []
            for p in range(NPAIR):
                n_full = znp.tile([128, Bc], bf16, tag="n", name=f"n_{t}_{p}")
                z_full = znp.tile([128, Bc], bf16, tag="z", name=f"z_{t}_{p}")
                for c in range(nch):
                    cs = slice(c * chunk, (c + 1) * chunk)
                    pr = ppr.tile([128, chunk], f32, tag="pr")
                    nc.tensor.matmul(pr[:], whl[0][p][:], h[p][:, cs], start=True, stop=False)
                    nc.tensor.matmul(pr[:], wxl[0][p][par][:], slab[k][:, cs], start=False, stop=True)
                    pz = ppz.tile([128, chunk], f32, tag="pz")
                    nc.tensor.matmul(pz[:], whl[1][p][:], h[p][:, cs], start=True, stop=False)
                    nc.tensor.matmul(pz[:], wxl[1][p][par][:], slab[k][:, cs], start=False, stop=True)
                    phn = pphn.tile([128, chunk], f32, tag="phn")
                    nc.tensor.matmul(phn[:], whl[2][p][:], h[p][:, cs], start=True, stop=True)
                    pxn = ppxn.tile([128, chunk], f32, tag="pxn")
                    nc.tensor.matmul(pxn[:], wxl[2][p][par][:], slab[k][:, cs], start=True, stop=True)

                    r_sb = gp.tile([128, chunk], bf16, tag="r")
                    nc.scalar.activation(r_sb[:], pr[:], AFT.Sigmoid, bias=bi_sb[0][p][:], scale=1.0)
                    nc.scalar.activation(z_full[:, cs], pz[:], AFT.Sigmoid, bias=bi_sb[1][p][:], scale=1.0)
                    t1 = gp.tile([128, chunk], bf16, tag="t1")
                    if bhn_zero:
                        nc.vector.tensor_mul(t1[:], r_sb[:], phn[:])
                    else:
                        nc.vector.scalar_tensor_tensor(
                            t1[:], phn[:], bhn_sb[p][:], r_sb[:],
                            op0=mybir.AluOpType.add, op1=mybir.AluOpType.mult,
                        )
                    npre = gp.tile([128, chunk], bf16, tag="npre")
                    nc.vector.tensor_add(npre[:], t1[:], pxn[:])
                    nc.scalar.activation(n_full[:, cs], npre[:], AFT.Tanh, bias=bi_sb[2][p][:], scale=1.0)

                hn_t = hp.tile([128, Bc], bf16, tag=f"h{p}", name=f"h_{t + 1}_{p}")
                for hh in range(2):
                    sl = slice(hh * half, (hh + 1) * half)
                    d = dep.tile([128, half], bf16, tag="d")
                    nc.gpsimd.tensor_sub(d[:], h[p][:, sl], n_full[:, sl])
                    e = dep.tile([128, half], bf16, tag="e")
                    nc.gpsimd.tensor_mul(e[:], d[:], z_full[:, sl])
                    nc.vector.tensor_add(hn_t[:, sl], n_full[:, sl], e[:])
                hnew.append(hn_t)
            h = hnew

        gru_psum.close()

        # ---- MLP head (feature-major) ----
        mp = ctx.enter_context(tc.tile_pool(name="mlp", bufs=1))
        h1 = [mp.tile([128, Bc], bf16, name=f"h1_{m}") for m in range(HID1 // 128)]
        h2 = [mp.tile([128, Bc], bf16, name=f"h2_{m}") for m in range(HID2 // 128)]
        ofm = mp.tile([M_OUT, Bc], f32, name="ofm")
        obt = mp.tile([128, nbt * M_OUT], f32, name="obt")
        pmp = ctx.enter_context(tc.tile_pool(name="pmp", bufs=4, space="PSUM"))
        pop = ctx.enter_context(tc.tile_pool(name="pop", bufs=2, space="PSUM"))
        ptp = ctx.enter_context(tc.tile_pool(name="ptp", bufs=2, space="PSUM"))

        for m in range(HID1 // 128):
            for c in range(nch):
                cs = slice(c * chunk, (c + 1) * chunk)
                pm = pmp.tile([128, chunk], f32, tag="pm")
                for kk in range(FEAT // 128):
                    nc.tensor.matmul(pm[:], w1l[kk][m][:], h[kk][:, cs],
                                     start=(kk == 0), stop=(kk == FEAT // 128 - 1))
                nc.scalar.activation(h1[m][:, cs], pm[:], AFT.Relu, bias=b1_sb[m][:], scale=1.0)
        for m in range(HID2 // 128):
            for c in range(nch):
                cs = slice(c * chunk, (c + 1) * chunk)
                pm = pmp.tile([128, chunk], f32, tag="pm")
                for kk in range(HID1 // 128):
                    nc.tensor.matmul(pm[:], w2l[kk][m][:], h1[kk][:, cs],
                                     start=(kk == 0), stop=(kk == HID1 // 128 - 1))
                nc.scalar.activation(h2[m][:, cs], pm[:], AFT.Relu, bias=b2_sb[m][:], scale=1.0)
        for c in range(nch):
            cs = slice(c * chunk, (c + 1) * chunk)
            po = pop.tile([M_OUT, chunk], f32, tag="po")
            for kk in range(HID2 // 128):
                nc.tensor.matmul(po[:], wol[kk][:], h2[kk][:, cs],
                                 start=(kk == 0), stop=(kk == HID2 // 128 - 1))
            nc.scalar.activation(ofm[:, cs], po[:], AFT.Sigmoid, bias=bout_sb[:], scale=1.0)

        # ---- transpose (15, B) -> (B, 15) and store ----
        for bt in range(nbt):
            pt = ptp.tile([128, M_OUT], f32, tag="pt")
            nc.tensor.transpose(pt[:], ofm[:, 128 * bt:128 * bt + 128], ident_sb[:])
            nc.scalar.copy(obt[:, M_OUT * bt:M_OUT * bt + M_OUT], pt[:])
        nc.sync.dma_start(
            out.rearrange("(bt p) f -> p bt f", p=128),
            obt[:].rearrange("p (bt f) -> p bt f", f=M_OUT),
        )

    nc.compile()
    return nc


def host_pack(inputs):
    """Pack weights into the on-device layouts (host-side, replicated)."""
    Wi = np.asarray(inputs["Wi"], np.float32)
    Wh = np.asarray(inputs["Wh"], np.float32)
    bi = np.asarray(inputs["bi"], np.float32)
    bhn = np.asarray(inputs["bhn"], np.float32)
    W1 = np.asarray(inputs["W1"], np.float32)
    b1 = np.asarray(inputs["b1"], np.float32)
    W2 = np.asarray(inputs["W2"], np.float32)
    b2 = np.asarray(inputs["b2"], np.float32)
    Wout = np.asarray(inputs["Wout"], np.float32)
    bout = np.asarray(inputs["bout"], np.float32)
    bf = ml_dtypes.bfloat16

    WHL = np.zeros((3, NPAIR, 128, 128), np.float32)
    WXL = np.zeros((3, NPAIR, 2, 128, 128), np.float32)
    BI = np.zeros((3, NPAIR, 128, 1), np.float32)
    BHN = np.zeros((NPAIR, 128, 1), np.float32)
    for g in range(3):
        for p in range(NPAIR):
            a, b = 2 * p, 2 * p + 1
            gs = slice(64 * g, 64 * g + 64)
            WHL[g, p, 0:64, 0:64] = Wh[a][:, gs]
            WHL[g, p, 64:128, 64:128] = Wh[b][:, gs]
            for q in range(2):
                r0 = 64 * q + 8 * p
                WXL[g, p, q, r0:r0 + 4, 0:64] = Wi[a][:, gs]
                WXL[g, p, q, r0 + 4:r0 + 8, 64:128] = Wi[b][:, gs]
            BI[g, p, 0:64, 0] = bi[a, gs]
            BI[g, p, 64:128, 0] = bi[b, gs]
    for p in range(NPAIR):
        BHN[p, 0:64, 0] = bhn[2 * p]
        BHN[p, 64:128, 0] = bhn[2 * p + 1]

    W1L = W1.reshape(FEAT // 128, 128, HID1 // 128, 128).transpose(0, 2, 1, 3)
    W2L = W2.reshape(HID1 // 128, 128, HID2 // 128, 128).transpose(0, 2, 1, 3)
    WOL = Wout.reshape(HID2 // 128, 128, M_OUT)

    return {
        "WHL": np.ascontiguousarray(WHL, dtype=bf),
        "WXL": np.ascontiguousarray(WXL, dtype=bf),
        "W1L": np.ascontiguousarray(W1L, dtype=bf),
        "W2L": np.ascontiguousarray(W2L, dtype=bf),
        "WOL": np.ascontiguousarray(WOL, dtype=bf),
        "BI": BI,
        "BHN": BHN,
        "B1": np.ascontiguousarray(b1.reshape(HID1 // 128, 128, 1)),
        "B2": np.ascontiguousarray(b2.reshape(HID2 // 128, 128, 1)),
        "BOUT": np.ascontiguousarray(bout.reshape(M_OUT, 1)),
        "IDT": np.eye(M_OUT, dtype=np.float32),
    }, bool(np.all(bhn == 0.0))


_CACHE = {}


def _get_nc(bhn_zero):
    key = ("nc", bhn_zero)
    if key not in _CACHE:
        _CACHE[key] = build_nc(bhn_zero=bhn_zero)
    return _CACHE[key]


def kernel(**inputs):
    from concourse.bass_utils import run_bass_kernel_spmd

    packed, bhn_zero = host_pack(inputs)
    nc = _get_nc(bhn_zero)
    xf = np.asarray(inputs["x"], np.float32)
    in_maps = [
        {"x": np.ascontiguousarray(xf[c * B:(c + 1) * B]), **packed}
        for c in range(N_CORES)
    ]
    res = run_bass_kernel_spmd(nc, in_maps, list(range(N_CORES)))
    return np.concatenate([r["out"] for r in res.results], axis=0).astype(np.float32)


# revision 19
# speedup vs baseline: 5946.8750x; 5946.8750x over previous
"""Trainium2 Bass kernel for PlayerSelectionNetwork (16-agent GRU + MLP head).

Strategy (8 NeuronCores, data-parallel over batch):
  - Each core processes B=2048 rows of x (16384/8).
  - Feature-major GRU: hidden state h kept as (features, batch) tiles, agents
    packed in pairs on the 128 SBUF partitions (agent 2p -> partitions 0:64,
    agent 2p+1 -> 64:128).
  - Gate matmuls are K=128 block-diagonal: lhsT holds both agents' recurrent
    weights on disjoint row/col blocks, so one matmul produces a pair-packed
    gate tile (r_a; r_b) directly in PSUM.
  - Input projections x@Wi are folded into the same PSUM accumulation via a
    second block-diagonal matmul against a feature-major x slab (two
    timesteps per 128-row slab).
  - Elementwise gate math runs pair-packed on ACT (sigmoid/tanh with fused
    per-partition bias) / DVE / GPSIMD, balanced across the three engines.
  - MLP head stays feature-major; final (15, B) activations are transposed
    to batch-major with the PE transpose and DMA'd out contiguously.

Weights are pre-packed on the host (block-diagonal layout, bf16) and shipped
as extra kernel inputs - they are tiny and replicated on all cores.
"""

import numpy as np
import ml_dtypes

# Model constants (match the reference problem definition).
B_FULL = 16384
N_CORES = 8
B = B_FULL // N_CORES  # per-core batch
T_OBS = 10
N_AGENTS = 16
INPUT_DIM = 4
H = 64
HID1 = 512
HID2 = 256
M_OUT = 15
F_IN = T_OBS * N_AGENTS * INPUT_DIM  # 640
FEAT = N_AGENTS * H  # 1024

CHUNK = 512  # moving free dim per matmul / psum tile
NPAIR = N_AGENTS // 2  # 8
NSLAB = F_IN // 128  # 5 (two timesteps per slab)


def build_nc(Bc=B, chunk=CHUNK, bhn_zero=True, t_steps=T_OBS, zn_bufs=4, g_bufs=4, de_bufs=4, interleave=False, psum_depth=1):
    """Build + compile the single-core Bass program (SPMD-replicated)."""
    import concourse.bacc as bacc
    import concourse.mybir as mybir
    import concourse.tile as tile
    from contextlib import ExitStack

    f32 = mybir.dt.float32
    bf16 = mybir.dt.bfloat16
    AFT = mybir.ActivationFunctionType

    nch = Bc // chunk
    half = Bc // 2
    nbt = Bc // 128

    nc = bacc.Bacc("TRN2", target_bir_lowering=False, debug=False)

    x = nc.dram_tensor("x", (Bc, F_IN), f32, kind="ExternalInput").ap()
    WHB = nc.dram_tensor("WHB", (128, 3 * NPAIR * 128), bf16, kind="ExternalInput").ap()
    WXB = nc.dram_tensor("WXB", (128, 3 * NPAIR * 2 * 128), bf16, kind="ExternalInput").ap()
    W1B = nc.dram_tensor("W1B", (128, (FEAT // 128) * (HID1 // 128) * 128), bf16, kind="ExternalInput").ap()
    W2B = nc.dram_tensor("W2B", (128, (HID1 // 128) * (HID2 // 128) * 128), bf16, kind="ExternalInput").ap()
    WOB = nc.dram_tensor("WOB", (128, (HID2 // 128) * M_OUT), bf16, kind="ExternalInput").ap()
    BIB = nc.dram_tensor("BIB", (128, 3 * NPAIR + NPAIR + HID1 // 128 + HID2 // 128), f32, kind="ExternalInput").ap()
    BOUT = nc.dram_tensor("BOUT", (M_OUT, 1), f32, kind="ExternalInput").ap()
    IDT = nc.dram_tensor("IDT", (M_OUT, M_OUT), f32, kind="ExternalInput").ap()
    ID128 = nc.dram_tensor("ID128", (128, 128), bf16, kind="ExternalInput").ap()
    xscr = nc.dram_tensor("xscr", (Bc, F_IN), bf16).ap()
    out = nc.dram_tensor("out", (Bc, M_OUT), f32, kind="ExternalOutput").ap()

    with tile.TileContext(nc) as tc, ExitStack() as ctx:
        # ---- persistent weight / bias tiles ----
        wp = ctx.enter_context(tc.tile_pool(name="weights", bufs=1))
        whb = wp.tile([128, 3 * NPAIR * 128], bf16, name="whb")
        wxb = wp.tile([128, 3 * NPAIR * 2 * 128], bf16, name="wxb")
        w1b = wp.tile([128, (FEAT // 128) * (HID1 // 128) * 128], bf16, name="w1b")
        w2b = wp.tile([128, (HID1 // 128) * (HID2 // 128) * 128], bf16, name="w2b")
        wob = wp.tile([128, (HID2 // 128) * M_OUT], bf16, name="wob")
        bib = wp.tile([128, 3 * NPAIR + NPAIR + HID1 // 128 + HID2 // 128], f32, name="bib")
        bout_sb = wp.tile([M_OUT, 1], f32, name="bout_sb")
        ident_sb = wp.tile([M_OUT, M_OUT], f32, name="ident_sb")
        id128 = wp.tile([128, 128], bf16, name="id128")
        nc.sync.dma_start(whb[:], WHB[:])
        nc.sync.dma_start(wxb[:], WXB[:])
        nc.sync.dma_start(w1b[:], W1B[:])
        nc.sync.dma_start(w2b[:], W2B[:])
        nc.sync.dma_start(wob[:], WOB[:])
        nc.sync.dma_start(bib[:], BIB[:])
        nc.sync.dma_start(bout_sb[:], BOUT[:])
        nc.sync.dma_start(ident_sb[:], IDT[:])
        nc.sync.dma_start(id128[:], ID128[:])
        c128 = lambda tile_, i: tile_[:, 128 * i:128 * (i + 1)]
        whl = [[c128(whb, g * NPAIR + p) for p in range(NPAIR)] for g in range(3)]
        wxl = [[[c128(wxb, (g * NPAIR + p) * 2 + q) for q in range(2)] for p in range(NPAIR)] for g in range(3)]
        w1l = [[c128(w1b, k * (HID1 // 128) + m) for m in range(HID1 // 128)] for k in range(FEAT // 128)]
        w2l = [[c128(w2b, k * (HID2 // 128) + m) for m in range(HID2 // 128)] for k in range(HID1 // 128)]
        wol = [wob[:, M_OUT * k:M_OUT * (k + 1)] for k in range(HID2 // 128)]
        bi_sb = [[bib[:, g * NPAIR + p:g * NPAIR + p + 1] for p in range(NPAIR)] for g in range(3)]
        bhn_sb = [bib[:, 3 * NPAIR + p:3 * NPAIR + p + 1] for p in range(NPAIR)]
        b1_sb = [bib[:, 4 * NPAIR + m:4 * NPAIR + m + 1] for m in range(HID1 // 128)]
        b2_sb = [bib[:, 4 * NPAIR + HID1 // 128 + m:4 * NPAIR + HID1 // 128 + m + 1] for m in range(HID2 // 128)]

        # ---- x -> feature-major bf16 slabs (two timesteps per slab) ----
        sp = ctx.enter_context(tc.tile_pool(name="slabs", bufs=1))
        slab = [sp.tile([128, Bc], bf16, name=f"slab{k}") for k in range(NSLAB)]
        QB = min(512, Bc)
        with tc.tile_pool(name="xstage", bufs=2) as xsp, tc.tile_pool(name="xbf", bufs=2) as xbp:
            for q in range(Bc // QB):
                qs = slice(q * QB, (q + 1) * QB)
                xf = xsp.tile([128, (QB // 128) * F_IN], f32, tag="xf", name=f"xf{q}")
                nc.sync.dma_start(
                    xf[:].rearrange("p (b f) -> p b f", f=F_IN),
                    x[qs].rearrange("(b p) f -> p b f", p=128),
                )
                xb = xbp.tile([128, (QB // 128) * F_IN], bf16, tag="xb", name=f"xb{q}")
                nc.vector.tensor_copy(xb[:], xf[:])
                nc.sync.dma_start(
                    xscr[qs].rearrange("(b p) f -> p b f", p=128),
                    xb[:].rearrange("p (b f) -> p b f", f=F_IN),
                )
                for k in range(NSLAB):
                    nc.sync.dma_start(
                        slab[k][:, qs],
                        xscr[qs, 128 * k:128 * k + 128],
                        transpose=True,
                    )

        # ---- GRU ----
        hp = ctx.enter_context(tc.tile_pool(name="h", bufs=1))
        gru_sbuf = ExitStack()
        znp = gru_sbuf.enter_context(tc.tile_pool(name="zn", bufs=zn_bufs))
        gp = gru_sbuf.enter_context(tc.tile_pool(name="gates", bufs=g_bufs))
        dep = gru_sbuf.enter_context(tc.tile_pool(name="de", bufs=de_bufs))
        gru_psum = ExitStack()
        ppr = gru_psum.enter_context(tc.tile_pool(name="ppr", bufs=psum_depth, space="PSUM"))
        ppz = gru_psum.enter_context(tc.tile_pool(name="ppz", bufs=psum_depth, space="PSUM"))
        pphn = gru_psum.enter_context(tc.tile_pool(name="pphn", bufs=psum_depth, space="PSUM"))
        ppxn = gru_psum.enter_context(tc.tile_pool(name="ppxn", bufs=psum_depth, space="PSUM"))

        PFD = (2 * chunk) // psum_depth  # psum free dim per tile
        ncp = Bc // PFD

        h = []
        for p in range(NPAIR):
            t0 = hp.tile([128, Bc], bf16, tag=f"h{p}", name=f"h_init{p}")
            nc.gpsimd.memset(t0[:], 0.0)
            h.append(t0)

        # Software-pipelined emission: engines are in-order, so interleave
        # phases of consecutive (t, p, cp) units to avoid head-of-line
        # blocking on the sigma -> mul/add -> tanh cross-engine chain.
        units = [(t, p, cp) for t in range(t_steps)
                 for p in range(NPAIR) for cp in range(ncp)]
        psums, npres, zf, nf, ws = {}, {}, {}, {}, {}

        def s0_matmuls(u):
            t, p, cp = u
            k, par = (t % T_OBS) // 2, (t % T_OBS) % 2
            pr = ppr.tile([128, PFD], f32, tag="pr")
            pz = ppz.tile([128, PFD], f32, tag="pz")
            phn = pphn.tile([128, PFD], f32, tag="phn")
            pxn = ppxn.tile([128, PFD], f32, tag="pxn")
            psums[u] = (pr, pz, phn, pxn)
            for cc in range(PFD // chunk):
                cs = slice(cp * PFD + cc * chunk, cp * PFD + (cc + 1) * chunk)
                ps = slice(cc * chunk, (cc + 1) * chunk)
                nc.tensor.matmul(pr[:, ps], whl[0][p][:], h[p][:, cs], start=True, stop=False)
                nc.tensor.matmul(pr[:, ps], wxl[0][p][par][:], slab[k][:, cs], start=False, stop=True)
                nc.tensor.matmul(pz[:, ps], whl[1][p][:], h[p][:, cs], start=True, stop=False)
                nc.tensor.matmul(pz[:, ps], wxl[1][p][par][:], slab[k][:, cs], start=False, stop=True)
                nc.tensor.matmul(phn[:, ps], whl[2][p][:], h[p][:, cs], start=True, stop=True)
                nc.tensor.matmul(pxn[:, ps], wxl[2][p][par][:], slab[k][:, cs], start=True, stop=False)

        def s1_gates(u):
            t, p, cp = u
            pr, pz, phn, pxn = psums[u]
            if (t, p) not in zf:
                zf[(t, p)] = znp.tile([128, Bc], bf16, tag="z", name=f"z_{t}_{p}")
                nf[(t, p)] = znp.tile([128, Bc], bf16, tag="n", name=f"n_{t}_{p}")
            cps = slice(cp * PFD, (cp + 1) * PFD)
            r_sb = gp.tile([128, PFD], bf16, tag="r")
            nc.scalar.activation(r_sb[:], pr[:], AFT.Sigmoid, bias=bi_sb[0][p][:], scale=1.0)
            nc.scalar.activation(zf[(t, p)][:, cps], pz[:], AFT.Sigmoid, bias=bi_sb[1][p][:], scale=1.0)
            t1 = gp.tile([128, PFD], bf16, tag="t1")
            if bhn_zero:
                nc.vector.tensor_mul(t1[:], r_sb[:], phn[:])
            else:
                nc.vector.scalar_tensor_tensor(
                    t1[:], phn[:], bhn_sb[p][:], r_sb[:],
                    op0=mybir.AluOpType.add, op1=mybir.AluOpType.mult,
                )
            for cc in range(PFD // chunk):
                ps = slice(cc * chunk, (cc + 1) * chunk)
                nc.tensor.matmul(pxn[:, ps], id128[:], t1[:, ps], start=False, stop=True)
            # w = z * h (off critical path; h still holds the old value here)
            w = dep.tile([128, PFD], bf16, tag="w")
            nc.gpsimd.tensor_mul(w[:], zf[(t, p)][:, cps], h[p][:, cps])
            ws[u] = w

        def s2_tanh(u):
            t, p, cp = u
            cps = slice(cp * PFD, (cp + 1) * PFD)
            pxn = psums.pop(u)[3]
            nc.scalar.activation(nf[(t, p)][:, cps], pxn[:],
                                 AFT.Tanh, bias=bi_sb[2][p][:], scale=1.0)
            # h' = w - (z - 1) * n  (= z*h + (1-z)*n), per chunk-pair
            up = gp.tile([128, PFD], bf16, tag="up")
            nc.vector.scalar_tensor_tensor(
                up[:], zf[(t, p)][:, cps], 1.0, nf[(t, p)][:, cps],
                op0=mybir.AluOpType.subtract, op1=mybir.AluOpType.mult,
            )
            nc.vector.tensor_sub(h[p][:, cps], ws.pop(u)[:], up[:])
            if cp == ncp - 1:
                nf.pop((t, p)); zf.pop((t, p))

        for i in range(len(units) + 2):
            if i < len(units):
                s0_matmuls(units[i])
            if 1 <= i <= len(units):
                s1_gates(units[i - 1])
            if 2 <= i <= len(units) + 1:
                s2_tanh(units[i - 2])

        gru_psum.close()
        gru_sbuf.close()

        # ---- MLP head (feature-major) ----
        mp = ctx.enter_context(tc.tile_pool(name="mlp", bufs=1))
        h1 = [mp.tile([128, Bc], bf16, name=f"h1_{m}") for m in range(HID1 // 128)]
        h2 = [mp.tile([128, Bc], bf16, name=f"h2_{m}") for m in range(HID2 // 128)]
        ofm = mp.tile([M_OUT, Bc], f32, name="ofm")
        obt = mp.tile([128, nbt * M_OUT], f32, name="obt")
        pmp = ctx.enter_context(tc.tile_pool(name="pmp", bufs=4, space="PSUM"))
        pop = ctx.enter_context(tc.tile_pool(name="pop", bufs=2, space="PSUM"))
        ptp = ctx.enter_context(tc.tile_pool(name="ptp", bufs=2, space="PSUM"))

        for m in range(HID1 // 128):
            pms = [pmp.tile([128, chunk], f32, tag="pm", name=f"pm1_{m}_{c}") for c in range(nch)]
            for kk in range(FEAT // 128):
                for c in range(nch):
                    cs = slice(c * chunk, (c + 1) * chunk)
                    nc.tensor.matmul(pms[c][:], w1l[kk][m][:], h[kk][:, cs],
                                     start=(kk == 0), stop=(kk == FEAT // 128 - 1))
            for c in range(nch):
                cs = slice(c * chunk, (c + 1) * chunk)
                nc.scalar.activation(h1[m][:, cs], pms[c][:], AFT.Relu, bias=b1_sb[m][:], scale=1.0)
        for m in range(HID2 // 128):
            pms = [pmp.tile([128, chunk], f32, tag="pm", name=f"pm2_{m}_{c}") for c in range(nch)]
            for kk in range(HID1 // 128):
                for c in range(nch):
                    cs = slice(c * chunk, (c + 1) * chunk)
                    nc.tensor.matmul(pms[c][:], w2l[kk][m][:], h1[kk][:, cs],
                                     start=(kk == 0), stop=(kk == HID1 // 128 - 1))
            for c in range(nch):
                cs = slice(c * chunk, (c + 1) * chunk)
                nc.scalar.activation(h2[m][:, cs], pms[c][:], AFT.Relu, bias=b2_sb[m][:], scale=1.0)
        for c in range(nch):
            cs = slice(c * chunk, (c + 1) * chunk)
            po = pop.tile([M_OUT, chunk], f32, tag="po")
            for kk in range(HID2 // 128):
                nc.tensor.matmul(po[:], wol[kk][:], h2[kk][:, cs],
                                 start=(kk == 0), stop=(kk == HID2 // 128 - 1))
            nc.scalar.activation(ofm[:, cs], po[:], AFT.Sigmoid, bias=bout_sb[:], scale=1.0)

        # ---- transpose (15, B) -> (B, 15) and store ----
        for bt in range(nbt):
            pt = ptp.tile([128, M_OUT], f32, tag="pt")
            nc.tensor.transpose(pt[:], ofm[:, 128 * bt:128 * bt + 128], ident_sb[:])
            nc.scalar.copy(obt[:, M_OUT * bt:M_OUT * bt + M_OUT], pt[:])
        nc.sync.dma_start(
            out.rearrange("(bt p) f -> p bt f", p=128),
            obt[:].rearrange("p (bt f) -> p bt f", f=M_OUT),
        )

    nc.compile()
    return nc


def host_pack(inputs):
    """Pack weights into SBUF-image layouts (one DMA per group on device)."""
    Wi = np.asarray(inputs["Wi"], np.float32)
    Wh = np.asarray(inputs["Wh"], np.float32)
    bi = np.asarray(inputs["bi"], np.float32)
    bhn = np.asarray(inputs["bhn"], np.float32)
    W1 = np.asarray(inputs["W1"], np.float32)
    b1 = np.asarray(inputs["b1"], np.float32)
    W2 = np.asarray(inputs["W2"], np.float32)
    b2 = np.asarray(inputs["b2"], np.float32)
    Wout = np.asarray(inputs["Wout"], np.float32)
    bout = np.asarray(inputs["bout"], np.float32)
    bf = ml_dtypes.bfloat16

    WHL = np.zeros((3, NPAIR, 128, 128), np.float32)
    WXL = np.zeros((3, NPAIR, 2, 128, 128), np.float32)
    for g in range(3):
        for p in range(NPAIR):
            a, b = 2 * p, 2 * p + 1
            gs = slice(64 * g, 64 * g + 64)
            WHL[g, p, 0:64, 0:64] = Wh[a][:, gs]
            WHL[g, p, 64:128, 64:128] = Wh[b][:, gs]
            for q in range(2):
                r0 = 64 * q + 8 * p
                WXL[g, p, q, r0:r0 + 4, 0:64] = Wi[a][:, gs]
                WXL[g, p, q, r0 + 4:r0 + 8, 64:128] = Wi[b][:, gs]
    WHB = WHL.transpose(2, 0, 1, 3).reshape(128, -1)
    WXB = WXL.transpose(3, 0, 1, 2, 4).reshape(128, -1)
    W1B = W1.reshape(FEAT // 128, 128, HID1 // 128, 128).transpose(1, 0, 2, 3).reshape(128, -1)
    W2B = W2.reshape(HID1 // 128, 128, HID2 // 128, 128).transpose(1, 0, 2, 3).reshape(128, -1)
    WOB = Wout.reshape(HID2 // 128, 128, M_OUT).transpose(1, 0, 2).reshape(128, -1)

    nb = 3 * NPAIR + NPAIR + HID1 // 128 + HID2 // 128
    BIB = np.zeros((128, nb), np.float32)
    for g in range(3):
        for p in range(NPAIR):
            BIB[0:64, g * NPAIR + p] = bi[2 * p, 64 * g:64 * g + 64]
            BIB[64:128, g * NPAIR + p] = bi[2 * p + 1, 64 * g:64 * g + 64]
    for p in range(NPAIR):
        BIB[0:64, 3 * NPAIR + p] = bhn[2 * p]
        BIB[64:128, 3 * NPAIR + p] = bhn[2 * p + 1]
    for m in range(HID1 // 128):
        BIB[:, 4 * NPAIR + m] = b1[128 * m:128 * m + 128]
    for m in range(HID2 // 128):
        BIB[:, 4 * NPAIR + HID1 // 128 + m] = b2[128 * m:128 * m + 128]

    return {
        "WHB": np.ascontiguousarray(WHB, dtype=bf),
        "WXB": np.ascontiguousarray(WXB, dtype=bf),
        "W1B": np.ascontiguousarray(W1B, dtype=bf),
        "W2B": np.ascontiguousarray(W2B, dtype=bf),
        "WOB": np.ascontiguousarray(WOB, dtype=bf),
        "BIB": BIB,
        "BOUT": np.ascontiguousarray(bout.reshape(M_OUT, 1)),
        "IDT": np.eye(M_OUT, dtype=np.float32),
        "ID128": np.eye(128, dtype=ml_dtypes.bfloat16),
    }, bool(np.all(bhn == 0.0))


_CACHE = {}


def _get_nc(bhn_zero):
    key = ("nc", bhn_zero)
    if key not in _CACHE:
        _CACHE[key] = build_nc(bhn_zero=bhn_zero)
    return _CACHE[key]


def kernel(**inputs):
    from concourse.bass_utils import run_bass_kernel_spmd

    packed, bhn_zero = host_pack(inputs)
    nc = _get_nc(bhn_zero)
    xf = np.asarray(inputs["x"], np.float32)
    in_maps = [
        {"x": np.ascontiguousarray(xf[c * B:(c + 1) * B]), **packed}
        for c in range(N_CORES)
    ]
    res = run_bass_kernel_spmd(nc, in_maps, list(range(N_CORES)))
    return np.concatenate([r["out"] for r in res.results], axis=0).astype(np.float32)
